# revision 19
# baseline (speedup 1.0000x reference)
"""Expert-parallel grouped-MLP (MoE experts) kernel for 8 Trainium2 cores.

Problem: y = W2_e @ silu(W1_e @ x_e + b1_e) + b2_e for E=16 independent
experts (grouped 1x1 conv), B=8 batches, C=256 channels/expert, CAP=4,
L=1024 positions. Expert-parallel: core i owns experts {2i, 2i+1}.

Speed trick ("linear hoist + single-pass fp8 residual path"):
  silu(z) = 0.5*z + g(z),  g = silu(z) - 0.5*z  (sigma_g ~ 0.45*sigma_h)
  y = W2@g + Wf@x + b2,    Wf := 0.5*(W2@W1)  (fused [C,C], host-exact)
The g-path runs as SINGLE fp8e4m3 DoubleRow matmuls (256-contraction per
pass -> 2x fp16 FLOP rate); g's small amplitude keeps the fp8
quantization error of both g and W2 inside the 2e-2 gate (measured
1.53e-2 on the fixed seed-0 inputs; plain fp8 h-path would be 5.3e-2).
The f-path and layer 1 stay fp16/exact. 28 512-col PE passes per
(pair, n-half) vs 32 for pure fp16.

Per (b, e) pair on-device:
  L1: per m-tile (8): 4 fp16 matmuls -> psum1 [128,1024] (= 0.5*z)
      ACT: h16 = silu(2*psum1 + b1)
      DVE/GpSimd (alternating): s8 = (h16 - 0.25) - psum1  -> fp8 (= g-0.25)
  L2: per (j,n): 2 fp16 Wf-matmuls + 4 fp8-DR W2g-matmuls -> psum2
      DVE: y16 = psum2/32 + b2'   (b2' = b2 + 0.25*rowsum(W2), host)
Host pre-scales: W1 x0.5 (psum holds 0.5z), W2g x32 fp8, Wf x16 fp16;
x ships fp16, y returns fp16 (upcast on host).
"""
import numpy as np
import ml_dtypes

import concourse.tile as tile
from concourse import bacc, mybir
from concourse.bass_utils import run_bass_kernel_spmd

# Problem constants (hardcoded per contract)
B, E, C, CAP, L = 8, 16, 256, 4, 1024
F = C * CAP            # 1024 hidden per expert
NCORES = 8
EPC = E // NCORES      # 2 experts per core
P = 128                # partitions
KT = C // P            # 2 fp16 k-tiles (layer-1 / f-path contraction)
KI = 2                 # DoubleRow k-interleave (256-contraction)
MT = F // P            # 8 m-tiles
JT = C // P            # 2 j-tiles
QT = F // (KI * P)     # 4 DoubleRow k-pairs (g-path contraction)
NT = L // 512          # 2 n-tiles of 512 cols
N_WARMUP = 8
SW = 32.0              # W2 scale
SHIFT = 0.25           # g mean shift (folded into b2')
GPS_M = 0              # m-tiles whose s8 runs via DVE-evac + GpSimd sub

_FP32 = mybir.dt.float32
_FP16 = mybir.dt.float16
_FP8 = mybir.dt.float8e4
_E4 = ml_dtypes.float8_e4m3


def _build():
    nc = bacc.Bacc("TRN2", target_bir_lowering=False, debug=False)
    DR = mybir.MatmulPerfMode.DoubleRow
    Silu = mybir.ActivationFunctionType.Silu
    Ident = mybir.ActivationFunctionType.Identity
    Sub = mybir.AluOpType.subtract
    Mult = mybir.AluOpType.mult
    Add = mybir.AluOpType.add

    # host layouts (contiguous per partition):
    #   xf[b, e, p, k, l] = fp16(x[b, e, k*128+p, l])
    #   w1[e, p, k, f]    = fp16(0.5 * W1r[e, f, k*128+p])
    #   wf[e, p, k, c]    = fp16(16 * (W2r@W1r)[e, c, k*128+p])
    #   w2[e, p, q, i, c] = fp8(32 * W2r[e, c, q*256+i*128+p])
    xs_d = nc.dram_tensor("xs", [B, EPC, P, KT, L], _FP16, kind="ExternalInput")
    w1_d = nc.dram_tensor("w1", [EPC, P, KT, F], _FP16, kind="ExternalInput")
    wf_d = nc.dram_tensor("wf", [EPC, P, KT, C], _FP16, kind="ExternalInput")
    w2_d = nc.dram_tensor("w2", [EPC, P, QT, KI, C], _FP8, kind="ExternalInput")
    b1_d = nc.dram_tensor("b1s", [EPC, F], _FP32, kind="ExternalInput")
    b2_d = nc.dram_tensor("b2s", [EPC, C], _FP32, kind="ExternalInput")
    ys_d = nc.dram_tensor("ys", [B, EPC * C, L], _FP16, kind="ExternalOutput")

    with tile.TileContext(nc) as tc:
        with (
            tc.tile_pool(name="const", bufs=1) as cpool,
            tc.tile_pool(name="x", bufs=6) as xpool,
            tc.tile_pool(name="h", bufs=2) as hpool,
            tc.tile_pool(name="y", bufs=4) as ypool,
            tc.tile_pool(name="ps1", bufs=3, space="PSUM") as ps1,
            tc.tile_pool(name="ps2", bufs=2, space="PSUM") as ps2,
        ):
            # ---- PE warmup: zero bf16 matmuls with no DMA deps ----
            wdum = cpool.tile([P, P], mybir.dt.bfloat16, tag="wdum")
            rdum = cpool.tile([P, 512], mybir.dt.bfloat16, tag="rdum")
            nc.vector.memset(wdum[:], 0.0)
            nc.vector.memset(rdum[:], 0.0)
            actdum = cpool.tile([P, 1], _FP32, tag="actdum")
            nc.scalar.activation(actdum[:], rdum[:, :1], Silu, bias=0.0)
            shiftc = cpool.tile([P, 1], _FP32, tag="shiftc")
            nc.vector.memset(shiftc[:], SHIFT)
            for i in range(N_WARMUP):
                pdum = ps2.tile([P, 512], _FP32, tag="ps2")
                nc.tensor.matmul(pdum[:, :128], wdum[:], rdum[:, :128],
                                 start=True, stop=True)

            # ---- weight/bias tiles ----
            w1sb = [cpool.tile([P, KT, F], _FP16, tag=f"w1_{e}",
                               name=f"w1sb_{e}") for e in range(EPC)]
            wfsb = [cpool.tile([P, KT, C], _FP16, tag=f"wf_{e}",
                               name=f"wfsb_{e}") for e in range(EPC)]
            w2sb = [cpool.tile([P, QT, KI, C], _FP8, tag=f"w2_{e}",
                               name=f"w2sb_{e}") for e in range(EPC)]
            b1sb = cpool.tile([P, EPC * MT], _FP32, tag="b1")  # col e*MT+m
            b2sb = cpool.tile([P, EPC * JT], _FP32, tag="b2")  # col e*JT+j

            def load_w(e):
                nc.sync.dma_start(w1sb[e][:], w1_d.ap()[e])
                nc.sync.dma_start(wfsb[e][:], wf_d.ap()[e])
                nc.sync.dma_start(w2sb[e][:], w2_d.ap()[e])

            def load_b(e):
                nc.sync.dma_start(
                    b1sb[:, e * MT:(e + 1) * MT],
                    b1_d.ap()[e].rearrange("(m p) -> p m", p=P),
                )
                nc.sync.dma_start(
                    b2sb[:, e * JT:(e + 1) * JT],
                    b2_d.ap()[e].rearrange("(j p) -> p j", p=P),
                )

            def load_x(b, e):
                # [P, KT, L] fp16; split per k-tile so mm0 waits on 2KB/part
                xt = xpool.tile([P, KT, L], _FP16, tag="x", name=f"x_{b}_{e}")
                for k in range(KT):
                    nc.sync.dma_start(xt[:, k], xs_d.ap()[b, e, :, k])
                return xt

            # startup-critical order (fine splits: mm0 waits on ~256KB)
            load_b(0)
            nc.sync.dma_start(w1sb[0][:, 0, 0:512], w1_d.ap()[0, :, 0, 0:512])
            x00 = xpool.tile([P, KT, L], _FP16, tag="x", name="x_0_0")
            nc.sync.dma_start(x00[:, 0, 0:512], xs_d.ap()[0, 0, :, 0, 0:512])
            nc.sync.dma_start(x00[:, 0, 512:L], xs_d.ap()[0, 0, :, 0, 512:L])
            nc.sync.dma_start(w1sb[0][:, 0, 512:F], w1_d.ap()[0, :, 0, 512:F])
            nc.sync.dma_start(x00[:, 1], xs_d.ap()[0, 0, :, 1])
            nc.sync.dma_start(w1sb[0][:, 1], w1_d.ap()[0, :, 1])
            nc.sync.dma_start(wfsb[0][:], wf_d.ap()[0])
            nc.sync.dma_start(w2sb[0][:], w2_d.ap()[0])
            x0 = x00

            def emit_l1(e, b, xt, h16, s8, m_lo, m_hi):
                for m in range(m_lo, m_hi):
                    psm = ps1.tile([P, L], _FP32, tag="ps1",
                                   name=f"ps1_{e}_{b}_{m}")
                    for k in range(KT):
                        for n in range(NT):
                            nc.tensor.matmul(
                                psm[:, n * 512:(n + 1) * 512],
                                w1sb[e][:, k, m * P:(m + 1) * P],
                                xt[:, k, n * 512:(n + 1) * 512],
                                start=(k == 0), stop=(k == KT - 1))
                    nc.scalar.activation(
                        h16[:, m], psm[:], Silu,
                        bias=b1sb[:, e * MT + m: e * MT + m + 1],
                        scale=2.0)
                    # s8 = (h16 - SHIFT) - psum1   (= g - SHIFT, fp8)
                    nc.vector.scalar_tensor_tensor(
                        s8[:, m], h16[:, m], SHIFT, psm[:],
                        op0=Sub, op1=Sub)

            def emit_l2_open(e, b, xt, s8, j, q_hi):
                psy = [ps2.tile([P, 512], _FP32, tag="ps2",
                                name=f"ps2_{e}_{b}_{j}_{n}")
                       for n in range(NT)]
                for k in range(KT):
                    for n in range(NT):
                        nc.tensor.matmul(
                            psy[n][:],
                            wfsb[e][:, k, j * P:(j + 1) * P],
                            xt[:, k, n * 512:(n + 1) * 512],
                            start=(k == 0), stop=False)
                for q in range(q_hi):
                    for n in range(NT):
                        nc.tensor.matmul(
                            psy[n][:],
                            w2sb[e][:, q, :, j * P:(j + 1) * P],
                            s8[:, 2 * q:2 * q + 2, n * 512:(n + 1) * 512],
                            start=False, stop=False, perf_mode=DR)
                return psy

            def emit_l2_close(e, b, s8, j, psy, q_lo):
                for q in range(q_lo, QT):
                    for n in range(NT):
                        nc.tensor.matmul(
                            psy[n][:],
                            w2sb[e][:, q, :, j * P:(j + 1) * P],
                            s8[:, 2 * q:2 * q + 2, n * 512:(n + 1) * 512],
                            start=False, stop=(q == QT - 1),
                            perf_mode=DR)
                for n in range(NT):
                    yt = ypool.tile([P, 512], _FP16, tag="y",
                                    name=f"y_{e}_{b}_{j}_{n}")
                    if n == 0:
                        nc.scalar.activation(
                            yt[:], psy[n][:], Ident,
                            bias=b2sb[:, e * JT + j: e * JT + j + 1],
                            scale=1.0 / SW)
                    else:
                        nc.vector.tensor_scalar(
                            yt[:], psy[n][:], 1.0 / SW,
                            b2sb[:, e * JT + j: e * JT + j + 1],
                            op0=Mult, op1=Add)
                    nc.sync.dma_start(
                        ys_d.ap()[b, e * C + j * P: e * C + (j + 1) * P,
                                  n * 512:(n + 1) * 512],
                        yt[:])

            def emit_l2(e, b, xt, s8, j):
                psy = emit_l2_open(e, b, xt, s8, j, QT - 1)
                emit_l2_close(e, b, s8, j, psy, QT - 1)

            # ---- software-pipelined pair loop: L2(k-1) interleaves L1(k) ----
            prev = None
            npairs = EPC * B
            for idx in range(npairs):
                e, b = idx // B, idx % B
                last = idx == npairs - 1
                xt = x0 if idx == 0 else load_x(b, e)
                if idx == 1:
                    load_w(1)
                    load_b(1)
                h16 = hpool.tile([P, MT, L], _FP16, tag="h16",
                                 name=f"h16_{e}_{b}")
                s8 = hpool.tile([P, MT, L], _FP8, tag="s8",
                                name=f"s8_{e}_{b}")
                emit_l1(e, b, xt, h16, s8, 0, 4)
                if prev is not None:
                    emit_l2(prev[0], prev[1], prev[2], prev[3], 0)
                if not last:
                    emit_l1(e, b, xt, h16, s8, 4, MT)
                    if prev is not None:
                        emit_l2(prev[0], prev[1], prev[2], prev[3], 1)
                else:
                    # tail-shortening: open own j0 (Wf + q0,q1) behind L1
                    emit_l1(e, b, xt, h16, s8, 4, 6)
                    emit_l2(prev[0], prev[1], prev[2], prev[3], 1)
                    psy0 = emit_l2_open(e, b, xt, s8, 0, 2)
                    emit_l1(e, b, xt, h16, s8, 6, MT)
                    emit_l2_close(e, b, s8, 0, psy0, 2)
                    emit_l2(e, b, xt, s8, 1)
                prev = (e, b, xt, s8)

    nc.compile()
    return nc


_NC_CACHE = None


def _get_nc():
    global _NC_CACHE
    if _NC_CACHE is None:
        _NC_CACHE = _build()
    return _NC_CACHE


def _shard_inputs(x, W1, b1, W2, b2):
    """Full inputs -> list of 8 per-core input dicts (expert-parallel)."""
    x = np.ascontiguousarray(x, dtype=np.float32)
    # xf[b, e, p, k, l]
    xf = np.ascontiguousarray(
        x.reshape(B, E, KT, P, L).transpose(0, 1, 3, 2, 4).astype(np.float16))

    W1r = W1.astype(np.float32).reshape(E, F, C)
    W2r = W2.astype(np.float32).reshape(E, C, F)
    b1r = b1.astype(np.float32).reshape(E, F)
    b2r = b2.astype(np.float32).reshape(E, C)

    # w1[e, p, k, f] = 0.5 * W1r[e].T, fp16
    w1t = (0.5 * W1r).transpose(0, 2, 1).reshape(E, KT, P, F)
    w1s = np.ascontiguousarray(w1t.transpose(0, 2, 1, 3).astype(np.float16))
    # wf[e, p, k, c] = 16 * (W2r@W1r)[e].T, fp16
    wfr = 16.0 * np.einsum('ecf,efd->ecd', W2r, W1r, optimize=True)  # [E,C,C]
    wft = wfr.transpose(0, 2, 1).reshape(E, KT, P, C)
    wfs = np.ascontiguousarray(wft.transpose(0, 2, 1, 3).astype(np.float16))
    # w2[e, p, q, i, c] = fp8(32 * W2r[e].T)
    w2t = (SW * W2r).transpose(0, 2, 1).reshape(E, QT, KI, P, C)
    w2s = np.ascontiguousarray(w2t.transpose(0, 3, 1, 2, 4).astype(_E4))
    # b2' = b2 + SHIFT * rowsum(W2)
    b2p = np.ascontiguousarray(b2r + SHIFT * W2r.sum(axis=2))
    b1c = np.ascontiguousarray(b1r)

    in_maps = []
    for i in range(NCORES):
        es = slice(i * EPC, (i + 1) * EPC)
        in_maps.append({
            "xs": np.ascontiguousarray(xf[:, es]),
            "w1": np.ascontiguousarray(w1s[es]),
            "wf": np.ascontiguousarray(wfs[es]),
            "w2": np.ascontiguousarray(w2s[es]),
            "b1s": b1c[es],
            "b2s": b2p[es],
        })
    return in_maps


def run(x, W1, b1, W2, b2, trace=False, **trace_kwargs):
    nc = _get_nc()
    in_maps = _shard_inputs(x, W1, b1, W2, b2)
    res = run_bass_kernel_spmd(
        nc, in_maps, core_ids=list(range(NCORES)), trace=trace, **trace_kwargs
    )
    y = np.concatenate([res.results[i]["ys"] for i in range(NCORES)], axis=1)
    return y.astype(np.float32), res


def kernel(x, W1, b1, W2, b2):
    y, _ = run(x, W1, b1, W2, b2)
    return y


# revision 20
# speedup vs baseline: 1.0089x; 1.0089x over previous
"""Expert-parallel grouped-MLP (MoE experts) kernel for 8 Trainium2 cores.

Problem: y = W2_e @ silu(W1_e @ x_e + b1_e) + b2_e for E=16 independent
experts (grouped 1x1 conv), B=8 batches, C=256 channels/expert, CAP=4,
L=1024 positions. Expert-parallel: core i owns experts {2i, 2i+1}.

Speed trick ("linear hoist + single-pass fp8 residual path"):
  silu(z) = 0.5*z + g(z),  g = silu(z) - 0.5*z  (sigma_g ~ 0.45*sigma_h)
  y = W2@g + Wf@x + b2,    Wf := 0.5*(W2@W1)  (fused [C,C], host-exact)
The g-path runs as SINGLE fp8e4m3 DoubleRow matmuls (256-contraction per
pass -> 2x fp16 FLOP rate); g's small amplitude keeps the fp8
quantization error of both g and W2 inside the 2e-2 gate (measured
1.53e-2 on the fixed seed-0 inputs; plain fp8 h-path would be 5.3e-2).
The f-path and layer 1 stay fp16/exact. 28 512-col PE passes per
(pair, n-half) vs 32 for pure fp16.

Per (b, e) pair on-device:
  L1: per m-tile (8): 4 fp16 matmuls -> psum1 [128,1024] (= 0.5*z)
      ACT: h16 = silu(2*psum1 + b1)
      DVE/GpSimd (alternating): s8 = (h16 - 0.25) - psum1  -> fp8 (= g-0.25)
  L2: per (j,n): 2 fp16 Wf-matmuls + 4 fp8-DR W2g-matmuls -> psum2
      DVE: y16 = psum2/32 + b2'   (b2' = b2 + 0.25*rowsum(W2), host)
Host pre-scales: W1 x0.5 (psum holds 0.5z), W2g x32 fp8, Wf x16 fp16;
x ships fp16, y returns fp16 (upcast on host).
"""
import numpy as np
import ml_dtypes

import concourse.tile as tile
from concourse import bacc, mybir
from concourse.bass_utils import run_bass_kernel_spmd

# Problem constants (hardcoded per contract)
B, E, C, CAP, L = 8, 16, 256, 4, 1024
F = C * CAP            # 1024 hidden per expert
NCORES = 8
EPC = E // NCORES      # 2 experts per core
P = 128                # partitions
KT = C // P            # 2 fp16 k-tiles (layer-1 / f-path contraction)
KI = 2                 # DoubleRow k-interleave (256-contraction)
MT = F // P            # 8 m-tiles
JT = C // P            # 2 j-tiles
QT = F // (KI * P)     # 4 DoubleRow k-pairs (g-path contraction)
NT = L // 512          # 2 n-tiles of 512 cols
N_WARMUP = 16
SW = 32.0              # W2 scale
SHIFT = 0.25           # g mean shift (folded into b2')
GPS_M = 0              # m-tiles whose s8 runs via DVE-evac + GpSimd sub

_FP32 = mybir.dt.float32
_FP16 = mybir.dt.float16
_FP8 = mybir.dt.float8e4
_E4 = ml_dtypes.float8_e4m3


def _build():
    nc = bacc.Bacc("TRN2", target_bir_lowering=False, debug=False)
    DR = mybir.MatmulPerfMode.DoubleRow
    Silu = mybir.ActivationFunctionType.Silu
    Ident = mybir.ActivationFunctionType.Identity
    Sub = mybir.AluOpType.subtract
    Mult = mybir.AluOpType.mult
    Add = mybir.AluOpType.add

    # host layouts (contiguous per partition):
    #   xf[b, e, p, k, l] = fp16(x[b, e, k*128+p, l])
    #   w1[e, p, k, f]    = fp16(0.5 * W1r[e, f, k*128+p])
    #   wf[e, p, k, c]    = fp16(16 * (W2r@W1r)[e, c, k*128+p])
    #   w2[e, p, q, i, c] = fp8(32 * W2r[e, c, q*256+i*128+p])
    xs_d = nc.dram_tensor("xs", [B, EPC, P, KT, L], _FP16, kind="ExternalInput")
    w1_d = nc.dram_tensor("w1", [EPC, P, KT, F], _FP16, kind="ExternalInput")
    wf_d = nc.dram_tensor("wf", [EPC, P, KT, C], _FP16, kind="ExternalInput")
    w2_d = nc.dram_tensor("w2", [EPC, P, QT, KI, C], _FP8, kind="ExternalInput")
    b1_d = nc.dram_tensor("b1s", [EPC, F], _FP32, kind="ExternalInput")
    b2_d = nc.dram_tensor("b2s", [EPC, C], _FP32, kind="ExternalInput")
    ys_d = nc.dram_tensor("ys", [B, EPC * C, L], _FP16, kind="ExternalOutput")

    with tile.TileContext(nc) as tc:
        with (
            tc.tile_pool(name="const", bufs=1) as cpool,
            tc.tile_pool(name="x", bufs=6) as xpool,
            tc.tile_pool(name="h", bufs=2) as hpool,
            tc.tile_pool(name="y", bufs=4) as ypool,
            tc.tile_pool(name="ps1", bufs=3, space="PSUM") as ps1,
            tc.tile_pool(name="ps2", bufs=2, space="PSUM") as ps2,
        ):
            # ---- PE warmup: zero bf16 matmuls with no DMA deps ----
            wdum = cpool.tile([P, P], mybir.dt.bfloat16, tag="wdum")
            rdum = cpool.tile([P, 512], mybir.dt.bfloat16, tag="rdum")
            nc.vector.memset(wdum[:], 0.0)
            nc.vector.memset(rdum[:], 0.0)
            actdum = cpool.tile([P, 1], _FP32, tag="actdum")
            nc.scalar.activation(actdum[:], rdum[:, :1], Silu, bias=0.0)
            shiftc = cpool.tile([P, 1], _FP32, tag="shiftc")
            nc.vector.memset(shiftc[:], SHIFT)
            for i in range(N_WARMUP):
                pdum = ps2.tile([P, 512], _FP32, tag="ps2")
                nc.tensor.matmul(pdum[:], wdum[:], rdum[:],
                                 start=True, stop=True)

            # ---- weight/bias tiles ----
            w1sb = [cpool.tile([P, KT, F], _FP16, tag=f"w1_{e}",
                               name=f"w1sb_{e}") for e in range(EPC)]
            wfsb = [cpool.tile([P, KT, C], _FP16, tag=f"wf_{e}",
                               name=f"wfsb_{e}") for e in range(EPC)]
            w2sb = [cpool.tile([P, QT, KI, C], _FP8, tag=f"w2_{e}",
                               name=f"w2sb_{e}") for e in range(EPC)]
            b1sb = cpool.tile([P, EPC * MT], _FP32, tag="b1")  # col e*MT+m
            b2sb = cpool.tile([P, EPC * JT], _FP32, tag="b2")  # col e*JT+j

            def load_w(e):
                nc.sync.dma_start(w1sb[e][:], w1_d.ap()[e])
                nc.sync.dma_start(wfsb[e][:], wf_d.ap()[e])
                nc.sync.dma_start(w2sb[e][:], w2_d.ap()[e])

            def load_b(e):
                nc.sync.dma_start(
                    b1sb[:, e * MT:(e + 1) * MT],
                    b1_d.ap()[e].rearrange("(m p) -> p m", p=P),
                )
                nc.sync.dma_start(
                    b2sb[:, e * JT:(e + 1) * JT],
                    b2_d.ap()[e].rearrange("(j p) -> p j", p=P),
                )

            def load_x(b, e):
                # [P, KT, L] fp16; split per k-tile so mm0 waits on 2KB/part
                xt = xpool.tile([P, KT, L], _FP16, tag="x", name=f"x_{b}_{e}")
                for k in range(KT):
                    nc.sync.dma_start(xt[:, k], xs_d.ap()[b, e, :, k])
                return xt

            # startup-critical order (fine splits: mm0 waits on ~256KB)
            load_b(0)
            nc.sync.dma_start(w1sb[0][:, 0, 0:512], w1_d.ap()[0, :, 0, 0:512])
            x00 = xpool.tile([P, KT, L], _FP16, tag="x", name="x_0_0")
            nc.sync.dma_start(x00[:, 0, 0:512], xs_d.ap()[0, 0, :, 0, 0:512])
            nc.sync.dma_start(x00[:, 0, 512:L], xs_d.ap()[0, 0, :, 0, 512:L])
            nc.sync.dma_start(w1sb[0][:, 0, 512:F], w1_d.ap()[0, :, 0, 512:F])
            nc.sync.dma_start(x00[:, 1], xs_d.ap()[0, 0, :, 1])
            nc.sync.dma_start(w1sb[0][:, 1], w1_d.ap()[0, :, 1])
            nc.sync.dma_start(wfsb[0][:], wf_d.ap()[0])
            nc.sync.dma_start(w2sb[0][:], w2_d.ap()[0])
            x0 = x00

            def emit_l1(e, b, xt, h16, s8, m_lo, m_hi):
                for m in range(m_lo, m_hi):
                    psm = ps1.tile([P, L], _FP32, tag="ps1",
                                   name=f"ps1_{e}_{b}_{m}")
                    for k in range(KT):
                        for n in range(NT):
                            nc.tensor.matmul(
                                psm[:, n * 512:(n + 1) * 512],
                                w1sb[e][:, k, m * P:(m + 1) * P],
                                xt[:, k, n * 512:(n + 1) * 512],
                                start=(k == 0), stop=(k == KT - 1))
                    nc.scalar.activation(
                        h16[:, m], psm[:], Silu,
                        bias=b1sb[:, e * MT + m: e * MT + m + 1],
                        scale=2.0)
                    # s8 = (h16 - SHIFT) - psum1   (= g - SHIFT, fp8)
                    nc.vector.scalar_tensor_tensor(
                        s8[:, m], h16[:, m], SHIFT, psm[:],
                        op0=Sub, op1=Sub)

            def emit_l2_open(e, b, xt, s8, j, q_hi):
                psy = [ps2.tile([P, 512], _FP32, tag="ps2",
                                name=f"ps2_{e}_{b}_{j}_{n}")
                       for n in range(NT)]
                for k in range(KT):
                    for n in range(NT):
                        nc.tensor.matmul(
                            psy[n][:],
                            wfsb[e][:, k, j * P:(j + 1) * P],
                            xt[:, k, n * 512:(n + 1) * 512],
                            start=(k == 0), stop=False)
                for q in range(q_hi):
                    for n in range(NT):
                        nc.tensor.matmul(
                            psy[n][:],
                            w2sb[e][:, q, :, j * P:(j + 1) * P],
                            s8[:, 2 * q:2 * q + 2, n * 512:(n + 1) * 512],
                            start=False, stop=False, perf_mode=DR)
                return psy

            def emit_l2_close(e, b, s8, j, psy, q_lo):
                for q in range(q_lo, QT):
                    for n in range(NT):
                        nc.tensor.matmul(
                            psy[n][:],
                            w2sb[e][:, q, :, j * P:(j + 1) * P],
                            s8[:, 2 * q:2 * q + 2, n * 512:(n + 1) * 512],
                            start=False, stop=(q == QT - 1),
                            perf_mode=DR)
                for n in range(NT):
                    yt = ypool.tile([P, 512], _FP16, tag="y",
                                    name=f"y_{e}_{b}_{j}_{n}")
                    if n == 0:
                        nc.scalar.activation(
                            yt[:], psy[n][:], Ident,
                            bias=b2sb[:, e * JT + j: e * JT + j + 1],
                            scale=1.0 / SW)
                    else:
                        nc.vector.tensor_scalar(
                            yt[:], psy[n][:], 1.0 / SW,
                            b2sb[:, e * JT + j: e * JT + j + 1],
                            op0=Mult, op1=Add)
                    nc.sync.dma_start(
                        ys_d.ap()[b, e * C + j * P: e * C + (j + 1) * P,
                                  n * 512:(n + 1) * 512],
                        yt[:])

            def emit_l2(e, b, xt, s8, j):
                psy = emit_l2_open(e, b, xt, s8, j, QT - 1)
                emit_l2_close(e, b, s8, j, psy, QT - 1)

            # ---- software-pipelined pair loop: L2(k-1) interleaves L1(k) ----
            prev = None
            npairs = EPC * B
            for idx in range(npairs):
                e, b = idx // B, idx % B
                last = idx == npairs - 1
                xt = x0 if idx == 0 else load_x(b, e)
                if idx == 1:
                    load_w(1)
                    load_b(1)
                h16 = hpool.tile([P, MT, L], _FP16, tag="h16",
                                 name=f"h16_{e}_{b}")
                s8 = hpool.tile([P, MT, L], _FP8, tag="s8",
                                name=f"s8_{e}_{b}")
                emit_l1(e, b, xt, h16, s8, 0, 4)
                if prev is not None:
                    emit_l2(prev[0], prev[1], prev[2], prev[3], 0)
                if not last:
                    emit_l1(e, b, xt, h16, s8, 4, MT)
                    if prev is not None:
                        emit_l2(prev[0], prev[1], prev[2], prev[3], 1)
                else:
                    # tail-shortening: open own j0 (Wf + q0,q1) behind L1
                    emit_l1(e, b, xt, h16, s8, 4, 6)
                    emit_l2(prev[0], prev[1], prev[2], prev[3], 1)
                    psy0 = emit_l2_open(e, b, xt, s8, 0, 2)
                    emit_l1(e, b, xt, h16, s8, 6, MT)
                    emit_l2_close(e, b, s8, 0, psy0, 2)
                    emit_l2(e, b, xt, s8, 1)
                prev = (e, b, xt, s8)

    nc.compile()
    return nc


_NC_CACHE = None


def _get_nc():
    global _NC_CACHE
    if _NC_CACHE is None:
        _NC_CACHE = _build()
    return _NC_CACHE


def _shard_inputs(x, W1, b1, W2, b2):
    """Full inputs -> list of 8 per-core input dicts (expert-parallel)."""
    x = np.ascontiguousarray(x, dtype=np.float32)
    # xf[b, e, p, k, l]
    xf = np.ascontiguousarray(
        x.reshape(B, E, KT, P, L).transpose(0, 1, 3, 2, 4).astype(np.float16))

    W1r = W1.astype(np.float32).reshape(E, F, C)
    W2r = W2.astype(np.float32).reshape(E, C, F)
    b1r = b1.astype(np.float32).reshape(E, F)
    b2r = b2.astype(np.float32).reshape(E, C)

    # w1[e, p, k, f] = 0.5 * W1r[e].T, fp16
    w1t = (0.5 * W1r).transpose(0, 2, 1).reshape(E, KT, P, F)
    w1s = np.ascontiguousarray(w1t.transpose(0, 2, 1, 3).astype(np.float16))
    # wf[e, p, k, c] = 16 * (W2r@W1r)[e].T, fp16
    wfr = 16.0 * np.einsum('ecf,efd->ecd', W2r, W1r, optimize=True)  # [E,C,C]
    wft = wfr.transpose(0, 2, 1).reshape(E, KT, P, C)
    wfs = np.ascontiguousarray(wft.transpose(0, 2, 1, 3).astype(np.float16))
    # w2[e, p, q, i, c] = fp8(32 * W2r[e].T)
    w2t = (SW * W2r).transpose(0, 2, 1).reshape(E, QT, KI, P, C)
    w2s = np.ascontiguousarray(w2t.transpose(0, 3, 1, 2, 4).astype(_E4))
    # b2' = b2 + SHIFT * rowsum(W2)
    b2p = np.ascontiguousarray(b2r + SHIFT * W2r.sum(axis=2))
    b1c = np.ascontiguousarray(b1r)

    in_maps = []
    for i in range(NCORES):
        es = slice(i * EPC, (i + 1) * EPC)
        in_maps.append({
            "xs": np.ascontiguousarray(xf[:, es]),
            "w1": np.ascontiguousarray(w1s[es]),
            "wf": np.ascontiguousarray(wfs[es]),
            "w2": np.ascontiguousarray(w2s[es]),
            "b1s": b1c[es],
            "b2s": b2p[es],
        })
    return in_maps


def run(x, W1, b1, W2, b2, trace=False, **trace_kwargs):
    nc = _get_nc()
    in_maps = _shard_inputs(x, W1, b1, W2, b2)
    res = run_bass_kernel_spmd(
        nc, in_maps, core_ids=list(range(NCORES)), trace=trace, **trace_kwargs
    )
    y = np.concatenate([res.results[i]["ys"] for i in range(NCORES)], axis=1)
    return y.astype(np.float32), res


def kernel(x, W1, b1, W2, b2):
    y, _ = run(x, W1, b1, W2, b2)
    return y
